# revision 10
# baseline (speedup 1.0000x reference)
"""ConvCNP1d Trainium2 kernel.

Data-parallel over batch: 16 batches -> 8 cores x 2 batches.

The RBF kernels have support radius ~4.2 units (ls=ln2) on a 128-unit
domain, so K1[x,t] / K2[t,xt] are ~94% exact zeros. The host sorts xc
(with yc) and xt per batch; each 128-row block of sorted points then
only overlaps 1-2 of the four 512-column t/xt chunks. The program is
compiled for the union of needed (block, chunk) pairs across all
batches (same SPMD program on all cores; excluded pairs are < 1e-9
relative). This cuts exp/DVE/matmul work ~2.8x. The decoder output
lands in sorted-xt order and is unpermuted on the host.

Per (block, chunk) pair: the exponent d2 = t'^2 - 2x'*t' is one fused
DVE scalar_tensor_tensor op on a [128,512] tile (t' tables centered
per chunk, baked per column), the a*x'^2 term rides in as the Exp
activation bias, Exp writes the kernel tile in fp16, and one PE matmul
accumulates into the chunk's PSUM tile. The encoder phi has 33 columns
(col 0 = os_psi, col 32 = os_psi*yc) so h0 lands on partition 0 and h1
on partition 32 -- both legal compute-engine bases -- letting the h
epilogue run on PSUM directly: reciprocal_approx_fast on h0 (h0 >= ~6
for this data, eps irrelevant), h0 row scalar-copied straight into
rep2 row 0, ratio row DMA'd into row 1.

conv1d stack as tap-stacked fp16 matmuls: each layer's input is copied
4x on DVE (column-shifted by tap, partition bases 0/32/64/96) into a
shared [128, T] stack tile so taps 0-3 contract in ONE matmul; tap 4
reads the natural tile -- 2 matmuls per 512-chunk instead of 5.
conv1's t-row contribution is host-precomputed (TCONV). conv4 packs mu
at out-col 0 and sigma at out-col 32 (M=33); its sigma softplus
epilogue batches the Ln ops so the act table only swaps twice per
batch. os_rho folds into the K2 exp bias so mu/sigma need no
post-scale.
"""

import numpy as np

T_GRID = 2048
B = 16
N = 2048          # Nc == Nt == 2048
NCORES = 8
BLOC = B // NCORES
EPS = 1e-8
RSUP = 4.5        # RBF support radius in x units: exp(a*4.5^2) ~ 7e-10

_PROG_CACHE = {}


def build_program(enc_lists, dec_lists):
    import concourse.bacc as bacc
    import concourse.tile as tile
    from concourse import mybir

    f32 = mybir.dt.float32
    f16 = mybir.dt.float16
    AF = mybir.ActivationFunctionType
    # Bacc (not raw Bass): its compile() splits multi-sem waits into event
    # semaphores / ldweights, which the TRN2 ISA requires (1 wait per inst).
    nc = bacc.Bacc(None, target_bir_lowering=False)

    TPh = nc.declare_dram_parameter("TP_BC", [1, T_GRID], f32, isOutput=False)
    TSQh = nc.declare_dram_parameter("TSQ_BC", [1, T_GRID], f32, isOutput=False)
    XTPh = nc.declare_dram_parameter("XTP", [BLOC, T_GRID], f32, isOutput=False)
    XTSQh = nc.declare_dram_parameter("XTSQ", [BLOC, T_GRID], f32, isOutput=False)
    TS2h = nc.declare_dram_parameter("TS2", [BLOC, 128, 16, 4], f32, isOutput=False)
    TB2h = nc.declare_dram_parameter("TB2", [BLOC, 128, 16, 4], f32, isOutput=False)
    XS1h = nc.declare_dram_parameter("XS1", [BLOC, 128, 4, 16], f32, isOutput=False)
    XB1h = nc.declare_dram_parameter("XB1", [BLOC, 128, 4, 16], f32, isOutput=False)
    AVh = nc.declare_dram_parameter("AVEC", [128, 2], f32, isOutput=False)
    PHIh = nc.declare_dram_parameter("PHI", [BLOC, 128, 16 * 33], f16, isOutput=False)
    TCh = nc.declare_dram_parameter("TCONV", [16, T_GRID], f32, isOutput=False)
    W1Ah = nc.declare_dram_parameter("W1A", [128, 16], f16, isOutput=False)
    W1Bh = nc.declare_dram_parameter("W1B", [2, 16], f16, isOutput=False)
    W2Ah = nc.declare_dram_parameter("W2A", [128, 32], f16, isOutput=False)
    W2Bh = nc.declare_dram_parameter("W2B", [16, 32], f16, isOutput=False)
    W3Ah = nc.declare_dram_parameter("W3A", [128, 16], f16, isOutput=False)
    W3Bh = nc.declare_dram_parameter("W3B", [32, 16], f16, isOutput=False)
    W4Ah = nc.declare_dram_parameter("W4A", [128, 33], f16, isOutput=False)
    W4Bh = nc.declare_dram_parameter("W4B", [16, 33], f16, isOutput=False)
    B2h = nc.declare_dram_parameter("B2", [32, 1], f32, isOutput=False)
    B3h = nc.declare_dram_parameter("B3", [16, 1], f32, isOutput=False)
    Ch = nc.declare_dram_parameter("CONSTS", [2, 4], f32, isOutput=False)
    ID2h = nc.declare_dram_parameter("ID2", [2, 2], f16, isOutput=False)
    OUTh = nc.declare_dram_parameter("out", [BLOC, 2, T_GRID], f32, isOutput=True)

    with tile.TileContext(nc) as tc:
        with (
            tc.tile_pool(name="singles", bufs=1) as singles,
            tc.tile_pool(name="perb", bufs=2) as perb,
            tc.tile_pool(name="stacks", bufs=2) as stacks,
            tc.tile_pool(name="kpool", bufs=4) as kpool,
            tc.tile_pool(name="small", bufs=1) as small,
            tc.tile_pool(name="outs", bufs=2) as outs,
            tc.tile_pool(name="dvp", bufs=4) as dvp,
            tc.tile_pool(name="psd2", bufs=2, space="PSUM") as psd2,
            tc.tile_pool(name="psacc", bufs=4, space="PSUM") as psacc,
        ):
            import concourse.bass as bass_mod

            def bcast128(dst, src_ap):
                bc = bass_mod.AP(
                    tensor=src_ap.tensor, offset=src_ap.offset,
                    ap=[[0, 128], [1, T_GRID]],
                )
                nc.sync.dma_start(out=dst, in_=bc)

            TP_sb = singles.tile([128, T_GRID], f32)
            bcast128(TP_sb, TPh[:, :])
            TSQ_sb = singles.tile([128, T_GRID], f32)
            bcast128(TSQ_sb, TSQh[:, :])
            AV_sb = singles.tile([128, 2], f32)
            nc.sync.dma_start(out=AV_sb, in_=AVh[:, :])
            TC_sb = singles.tile([16, T_GRID], f32)
            nc.sync.dma_start(out=TC_sb, in_=TCh[:, :])
            W1A_sb = singles.tile([128, 16], f16)
            nc.sync.dma_start(out=W1A_sb, in_=W1Ah[:, :])
            W1B_sb = singles.tile([2, 16], f16)
            nc.sync.dma_start(out=W1B_sb, in_=W1Bh[:, :])
            W2A_sb = singles.tile([128, 32], f16)
            nc.sync.dma_start(out=W2A_sb, in_=W2Ah[:, :])
            W2B_sb = singles.tile([16, 32], f16)
            nc.sync.dma_start(out=W2B_sb, in_=W2Bh[:, :])
            W3A_sb = singles.tile([128, 16], f16)
            nc.sync.dma_start(out=W3A_sb, in_=W3Ah[:, :])
            W3B_sb = singles.tile([32, 16], f16)
            nc.sync.dma_start(out=W3B_sb, in_=W3Bh[:, :])
            W4A_sb = singles.tile([128, 33], f16)
            nc.sync.dma_start(out=W4A_sb, in_=W4Ah[:, :])
            W4B_sb = singles.tile([16, 33], f16)
            nc.sync.dma_start(out=W4B_sb, in_=W4Bh[:, :])
            B2_sb = singles.tile([32, 1], f32)
            nc.sync.dma_start(out=B2_sb, in_=B2h[:, :])
            B3_sb = singles.tile([16, 1], f32)
            nc.sync.dma_start(out=B3_sb, in_=B3h[:, :])
            C_sb = singles.tile([2, 4], f32)
            nc.sync.dma_start(out=C_sb, in_=Ch[:, :])
            ID2_sb = singles.tile([2, 2], f16)
            nc.sync.dma_start(out=ID2_sb, in_=ID2h[:, :])

            st = [dict() for _ in range(BLOC)]  # per-batch tile handles

            def loads(b):
                s = st[b]
                s["XS1"] = perb.tile([128, 4, 16], f32, tag="XS1", name="XS1_sb")
                nc.sync.dma_start(out=s["XS1"], in_=XS1h[b])
                s["XB1"] = perb.tile([128, 4, 16], f32, tag="XB1", name="XB1_sb")
                nc.sync.dma_start(out=s["XB1"], in_=XB1h[b])
                s["TS2"] = perb.tile([128, 16, 4], f32, tag="TS2", name="TS2_sb")
                nc.sync.dma_start(out=s["TS2"], in_=TS2h[b])
                s["TB2"] = perb.tile([128, 16, 4], f32, tag="TB2", name="TB2_sb")
                nc.sync.dma_start(out=s["TB2"], in_=TB2h[b])
                xtp = perb.tile([128, T_GRID], f32, tag="xtp", name="xtp")
                xsrc = XTPh[b]
                nc.sync.dma_start(out=xtp, in_=bass_mod.AP(
                    tensor=xsrc.tensor, offset=xsrc.offset,
                    ap=[[0, 128], [1, T_GRID]]))
                s["xtp"] = xtp
                xtsq = perb.tile([128, T_GRID], f32, tag="xtsq", name="xtsq")
                qsrc = XTSQh[b]
                nc.sync.dma_start(out=xtsq, in_=bass_mod.AP(
                    tensor=qsrc.tensor, offset=qsrc.offset,
                    ap=[[0, 128], [1, T_GRID]]))
                s["xtsq"] = xtsq
                s["PHI"] = perb.tile([128, 16 * 33], f16, tag="PHI", name="PHI_sb")
                nc.sync.dma_start(out=s["PHI"], in_=PHIh[b])
                rep2 = perb.tile([2, T_GRID + 4], f16, tag="rep2", name="rep2")
                nc.vector.memset(rep2[:, 0:2], 0.0)
                nc.vector.memset(rep2[:, T_GRID + 2 : T_GRID + 4], 0.0)
                s["rep2"] = rep2

            def stage_a(b):
                s = st[b]
                h_ps = {}
                kq = []
                seq = []
                for c in range(4):
                    blocks = enc_lists[c]
                    for idx, i in enumerate(blocks):
                        seq.append((c, i, idx == 0, idx == len(blocks) - 1))

                def gen_enc(k):
                    c, i, first, last = seq[k]
                    sl = slice(512 * c, 512 * (c + 1))
                    d2s = dvp.tile([128, 512], f32, tag="d2s", name="d2s")
                    nc.vector.scalar_tensor_tensor(
                        d2s,
                        TP_sb[:, sl],
                        s["XS1"][:, c, i : i + 1],
                        TSQ_sb[:, sl],
                        mybir.AluOpType.mult,
                        mybir.AluOpType.add,
                    )
                    K1 = kpool.tile([128, 512], f16, tag="K", name="K1")
                    nc.scalar.activation(
                        out=K1, in_=d2s, func=AF.Exp,
                        scale=AV_sb[:, 0:1], bias=s["XB1"][:, c, i : i + 1],
                    )
                    kq.append((K1, c, i, first, last))

                def acc_enc():
                    K1, c, i, first, last = kq.pop(0)
                    if first:
                        h_ps[c] = psacc.tile([33, 512], f32, tag="acc", name="h_acc")
                    nc.tensor.matmul(
                        h_ps[c],
                        s["PHI"][:, 33 * i : 33 * i + 33],
                        K1,
                        start=first,
                        stop=last,
                    )
                    if last:
                        # h0 on partition 0, h1 on partition 32; epilogue runs
                        # on PSUM directly. h0 >= ~6 so no eps guard needed
                        # before the reciprocal.
                        sl2 = slice(2 + 512 * c, 2 + 512 * (c + 1))
                        rec = small.tile([1, 512], f32, tag=f"rec{c}", name="rec")
                        ratf = small.tile([1, 512], f16, tag=f"rat{c}", name="ratf")
                        nc.vector.reciprocal_approx_fast(
                            out=rec, in_=h_ps[c][0:1, :])
                        nc.scalar.copy(s["rep2"][0:1, sl2], h_ps[c][0:1, :])
                        nc.vector.tensor_mul(ratf, h_ps[c][32:33, :], rec)
                        nc.sync.dma_start(out=s["rep2"][1:2, sl2], in_=ratf)

                for k in range(len(seq) + 1):
                    if k < len(seq):
                        gen_enc(k)
                    if k >= 1:
                        acc_enc()

            def stack_copies(b, src, nrows, memset_first=False):
                """Copy src rows 4x on DVE (column-shifted by tap o, partition
                base 32*o) into the shared [128, T] stack tile. Layers of a
                batch are serially dependent, so slot rotation across the two
                batches is the only concurrency needed."""
                stk = stacks.tile([128, T_GRID], f16, tag="stk", name="stk")
                if memset_first:
                    # unused partition rows must hold finite values (they get
                    # multiplied by zero weights); pool slots recycle our own
                    # f16 data after conv1, but its first use is raw SBUF.
                    nc.gpsimd.memset(stk, 0.0)
                for o in range(4):
                    nc.gpsimd.tensor_copy(
                        stk[32 * o : 32 * o + nrows, :],
                        src[0:nrows, o : o + T_GRID],
                    )
                return stk

            def stage_b_layer(b, l):
                """conv layer l for batch b: taps 0-3 contract in one K=128
                matmul against the stack tile; tap 4 reads the natural tile."""
                s = st[b]
                if l == 0:
                    for nmt, shp in (("f1", 16), ("f2", 32), ("f3", 16)):
                        s[nmt] = perb.tile([shp, T_GRID + 4], f16, tag=nmt, name=nmt)
                        nc.vector.memset(s[nmt][:, 0:2], 0.0)
                        nc.vector.memset(s[nmt][:, T_GRID + 2 : T_GRID + 4], 0.0)
                    s["fmu"] = perb.tile([1, T_GRID], f16, tag="fmu_r", name="fmu_r")
                    s["fsg"] = perb.tile([1, T_GRID], f16, tag="fsg_r", name="fsg_r")

                if l == 0:
                    # conv1: stack from rep2 (2 rows); the t-row term + b1 is
                    # the host-precomputed TCONV, added on DVE before relu.
                    stk = stack_copies(b, s["rep2"], 2, memset_first=True)
                    for n in range(4):
                        ps = psacc.tile([16, 512], f32, tag="acc", name="c1ps")
                        nc.tensor.matmul(
                            ps, W1A_sb, stk[:, 512 * n : 512 * n + 512],
                            start=True, stop=False,
                        )
                        nc.tensor.matmul(
                            ps, W1B_sb,
                            s["rep2"][:, 512 * n + 4 : 512 * n + 516],
                            start=False, stop=True,
                        )
                        nc.vector.tensor_add(
                            ps, ps, TC_sb[:, 512 * n : 512 * (n + 1)]
                        )
                        nc.scalar.activation(
                            out=s["f1"][:, 2 + 512 * n : 2 + 512 * (n + 1)],
                            in_=ps,
                            func=AF.Relu,
                        )
                elif l in (1, 2):
                    in_tile, wa, wb, bias_sb, O, nrows = (
                        (s["f1"], W2A_sb, W2B_sb, B2_sb, 32, 16) if l == 1
                        else (s["f2"], W3A_sb, W3B_sb, B3_sb, 16, 32)
                    )
                    out_tile = s["f2"] if l == 1 else s["f3"]
                    stk = stack_copies(b, in_tile, nrows)
                    for n in range(4):
                        ps = psacc.tile([O, 512], f32, tag="acc", name="cps")
                        nc.tensor.matmul(
                            ps, wa, stk[:, 512 * n : 512 * n + 512],
                            start=True, stop=False,
                        )
                        nc.tensor.matmul(
                            ps, wb,
                            in_tile[:, 512 * n + 4 : 512 * n + 516],
                            start=False, stop=True,
                        )
                        nc.scalar.activation(
                            out=out_tile[:, 2 + 512 * n : 2 + 512 * (n + 1)],
                            in_=ps,
                            func=AF.Relu,
                            bias=bias_sb,
                        )
                else:
                    # conv4: mu at out-col 0, sigma at out-col 32 (M=33) so one
                    # matmul pair produces both at legal partition bases. The
                    # sigma softplus epilogue batches Exp then Ln across chunks
                    # so the act table only swaps twice per batch:
                    # softplus(x+b) = relu(x+b) + ln(1 + exp(-|x+b|)).
                    stk = stack_copies(b, s["f3"], 16)
                    sas, srs = [], []
                    for n in range(4):
                        ps = psacc.tile([33, 512], f32, tag="acc", name="c4ps")
                        nc.tensor.matmul(
                            ps, W4A_sb, stk[:, 512 * n : 512 * n + 512],
                            start=True, stop=False,
                        )
                        nc.tensor.matmul(
                            ps, W4B_sb,
                            s["f3"][:, 512 * n + 4 : 512 * n + 516],
                            start=False, stop=True,
                        )
                        sl = slice(512 * n, 512 * (n + 1))
                        sa = small.tile([1, 512], f32, tag=f"sa{n}", name="sa")
                        sr = small.tile([1, 512], f32, tag=f"sr{n}", name="sr")
                        nc.scalar.activation(
                            out=s["fmu"][0:1, sl], in_=ps[0:1, :],
                            func=AF.Identity, bias=C_sb[0:1, 0:1],
                        )
                        nc.scalar.activation(
                            out=sa, in_=ps[32:33, :], func=AF.Abs,
                            bias=C_sb[0:1, 1:2],
                        )
                        nc.scalar.activation(
                            out=sr, in_=ps[32:33, :], func=AF.Relu,
                            bias=C_sb[0:1, 1:2],
                        )
                        sas.append(sa)
                        srs.append(sr)
                    for sa in sas:
                        nc.scalar.activation(out=sa, in_=sa, func=AF.Exp, scale=-1.0)
                    for sa in sas:
                        nc.scalar.activation(out=sa, in_=sa, func=AF.Ln, bias=1.0)
                    for n in range(4):
                        sl = slice(512 * n, 512 * (n + 1))
                        nc.gpsimd.tensor_add(s["fsg"][0:1, sl], sas[n], srs[n])

            def stage_t(b):
                # transpose fmu/fsg rows -> fT[p, c, j] = f'_c[128j+p]; the
                # PSUM->SBUF copies alternate scalar/DVE to split the cost.
                s = st[b]
                fT = perb.tile([128, 2, 16], f16, tag="fT", name="fT")
                s["fT"] = fT
                for j in range(16):
                    for c, row in enumerate((s["fmu"], s["fsg"])):
                        tp = psd2.tile([128, 1], f16, tag="d2", name="tp")
                        nc.tensor.transpose(
                            tp, row[0:1, 128 * j : 128 * (j + 1)], ID2_sb[0:1, 0:1]
                        )
                        if (j + c) % 2 == 0:
                            nc.scalar.copy(fT[:, c : c + 1, j], tp)
                        else:
                            nc.vector.tensor_copy(fT[:, c : c + 1, j], tp)

            def stage_c(b):
                s = st[b]
                fT = s["fT"]
                ms_ps = {}
                kq2 = []
                seq = []
                for q in range(4):
                    blocks = dec_lists[q]
                    for idx, j in enumerate(blocks):
                        seq.append((q, j, idx == 0, idx == len(blocks) - 1))

                def gen_dec(k):
                    q, j, first, last = seq[k]
                    sl = slice(512 * q, 512 * (q + 1))
                    d2s = dvp.tile([128, 512], f32, tag="d2s", name="d2c")
                    nc.vector.scalar_tensor_tensor(
                        d2s,
                        s["xtp"][:, sl],
                        s["TS2"][:, j, q : q + 1],
                        s["xtsq"][:, sl],
                        mybir.AluOpType.mult,
                        mybir.AluOpType.add,
                    )
                    K2 = kpool.tile([128, 512], f16, tag="K", name="K2")
                    nc.scalar.activation(
                        out=K2, in_=d2s, func=AF.Exp,
                        scale=AV_sb[:, 1:2], bias=s["TB2"][:, j, q : q + 1],
                    )
                    kq2.append((K2, q, j, first, last))

                def acc_dec():
                    K2, q, j, first, last = kq2.pop(0)
                    if first:
                        ms_ps[q] = psacc.tile([2, 512], f32, tag="acc", name="ms_acc")
                    nc.tensor.matmul(
                        ms_ps[q],
                        fT[:, :, j],
                        K2,
                        start=first,
                        stop=last,
                    )
                    if last:
                        ms_sb = outs.tile([2, 512], f32, tag="ms_sb", name="ms_sb")
                        nc.vector.tensor_copy(ms_sb, ms_ps[q][:, :])
                        nc.sync.dma_start(
                            out=OUTh[b, :, 512 * q : 512 * (q + 1)],
                            in_=ms_sb,
                        )

                for k in range(len(seq) + 1):
                    if k < len(seq):
                        gen_dec(k)
                    if k >= 1:
                        acc_dec()

            loads(0)
            loads(1)
            stage_a(0)
            stage_a(1)
            for l in range(4):
                for b in range(BLOC):
                    stage_b_layer(b, l)
            stage_t(0)
            stage_c(0)
            stage_t(1)
            stage_c(1)

    nc.compile()
    return nc


def make_inmaps(inputs):
    """Host-side sorting, window structure, and table construction.
    Returns (list of 8 per-core input dicts); stores the compiled-program
    structure and output permutations in _PROG_CACHE."""
    f32 = np.float32
    f16 = np.float16
    f64 = np.float64
    xc = np.asarray(inputs["xc"])[..., 0].astype(f32)
    yc = np.asarray(inputs["yc"])[..., 0].astype(f32)
    xt = np.asarray(inputs["xt"])[..., 0].astype(f32)
    ls_psi = f64(np.float32(inputs["ls_psi"]))
    os_psi = f64(np.float32(inputs["os_psi"]))
    ls_rho = f64(np.float32(inputs["ls_rho"]))
    os_rho = f64(np.float32(inputs["os_rho"]))
    w = [np.asarray(inputs[f"w{i}"]).astype(f32) for i in (1, 2, 3, 4)]
    bs = [np.asarray(inputs[f"b{i}"]).astype(f32) for i in (1, 2, 3, 4)]

    lower = np.minimum(xc.min(), xt.min())
    upper = np.maximum(xc.max(), xt.max())
    t64 = np.linspace(f64(lower), f64(upper), T_GRID)

    a_psi = -0.5 / (ls_psi * ls_psi)
    a_rho = -0.5 / (ls_rho * ls_rho)

    # sort context/target points per batch
    perm_c = np.argsort(xc, axis=1)
    perm_t = np.argsort(xt, axis=1)
    xs = np.take_along_axis(xc.astype(f64), perm_c, 1)
    ys = np.take_along_axis(yc.astype(f64), perm_c, 1)
    xts = np.take_along_axis(xt.astype(f64), perm_t, 1)

    # window structure: union over batches of needed (block, chunk) pairs
    ch_lo = t64[np.arange(4) * 512]
    ch_hi = t64[np.arange(4) * 512 + 511]
    enc_sets = [set() for _ in range(4)]
    dec_sets = [set() for _ in range(4)]
    for b in range(B):
        for i in range(16):
            blo, bhi = xs[b, 128 * i], xs[b, 128 * i + 127]
            for c in range(4):
                if bhi + RSUP >= ch_lo[c] and blo - RSUP <= ch_hi[c]:
                    enc_sets[c].add(i)
        for q in range(4):
            qlo, qhi = xts[b, 512 * q], xts[b, 512 * q + 511]
            for j in range(16):
                tlo, thi = t64[128 * j], t64[128 * j + 127]
                if thi + RSUP >= qlo and tlo - RSUP <= qhi:
                    dec_sets[q].add(j)
    enc_lists = tuple(tuple(sorted(sset)) for sset in enc_sets)
    dec_lists = tuple(tuple(sorted(sset)) for sset in dec_sets)
    _PROG_CACHE["struct"] = (enc_lists, dec_lists)
    _PROG_CACHE["perm_t"] = perm_t

    # t' tables centered per 512-chunk, baked per column
    cC = (t64[np.arange(4) * 512] + t64[np.arange(4) * 512 + 511]) / 2
    tp_col = t64 - cC[np.arange(T_GRID) // 512]
    TP = tp_col.astype(f32)[None, :]
    TSQ = (tp_col * tp_col).astype(f32)[None, :]
    AVEC = np.zeros((128, 2), f32)
    AVEC[:, 0] = f32(a_psi)
    AVEC[:, 1] = f32(a_rho)
    ln_os_rho = np.log(os_rho)

    XS1 = np.zeros((B, 128, 4, 16), f32)
    XB1 = np.zeros((B, 128, 4, 16), f32)
    PHI = np.zeros((B, 128, 16, 33), f32)
    TS2 = np.zeros((B, 128, 16, 4), f32)
    TB2 = np.zeros((B, 128, 16, 4), f32)
    XTP = np.zeros((B, T_GRID), f32)
    XTSQ = np.zeros((B, T_GRID), f32)
    tb = t64.reshape(16, 128)                       # [j, p]
    for bi in range(B):
        xcb = xs[bi].reshape(16, 128)               # [i, p]
        for c in range(4):
            xp = xcb - cC[c]
            XS1[bi, :, c, :] = (-2.0 * xp).astype(f32).T
            XB1[bi, :, c, :] = (a_psi * xp * xp).astype(f32).T
        phi_full = np.stack([np.full(N, os_psi), os_psi * ys[bi]], 1)
        pf = phi_full.astype(f32).reshape(16, 128, 2).transpose(1, 0, 2)
        PHI[bi, :, :, 0] = pf[:, :, 0]
        PHI[bi, :, :, 32] = pf[:, :, 1]
        cQ = (xts[bi, np.arange(4) * 512] + xts[bi, np.arange(4) * 512 + 511]) / 2
        for q in range(4):
            tpq = tb - cQ[q]                        # [j, p]
            TS2[bi, :, :, q] = (-2.0 * tpq).astype(f32).T
            TB2[bi, :, :, q] = (a_rho * tpq * tpq + ln_os_rho).astype(f32).T
        xtp_col = xts[bi] - cQ[np.arange(T_GRID) // 512]
        XTP[bi] = xtp_col.astype(f32)
        XTSQ[bi] = (xtp_col * xtp_col).astype(f32)

    # TCONV[o, t] = sum_o' w1[o, 0, o'] * t_pad[t + o'] + b1[o]  (exact fp64)
    t_pad = np.zeros(T_GRID + 4, f64)
    t_pad[2 : 2 + T_GRID] = t64
    TCONV = np.zeros((16, T_GRID), f64)
    for o in range(5):
        TCONV += w[0][:, 0, o].astype(f64)[:, None] * t_pad[o : o + T_GRID][None, :]
    TCONV += bs[0].astype(f64)[:, None]

    def pack_stack(wl, rows):
        # A: [128, O]: partition 32*o + i = wl[j, rows[i], o] for tap o<4
        # B: [I', O]: tap 4
        O, _, _ = wl.shape
        wr = wl[:, rows, :]                         # [O, I', 5]
        I2 = wr.shape[1]
        A = np.zeros((128, O), f32)
        Bm = np.zeros((I2, O), f32)
        for o in range(4):
            A[32 * o : 32 * o + I2, :] = wr[:, :, o].T
        Bm[:, :] = wr[:, :, 4].T
        return A.astype(f16), Bm.astype(f16)

    W1A, W1B = pack_stack(w[0], slice(1, 3))        # I'=2, O=16
    W2A, W2B = pack_stack(w[1], slice(None))        # I'=16, O=32
    W3A, W3B = pack_stack(w[2], slice(None))        # I'=32, O=16
    # conv4: mu weights at out-col 0, sigma at out-col 32
    w4m = np.zeros((33, 16, 5), f32)
    w4m[0] = w[3][0]
    w4m[32] = w[3][1]
    W4A = np.zeros((128, 33), f32)
    W4B = np.zeros((16, 33), f32)
    for o in range(4):
        W4A[32 * o : 32 * o + 16, :] = w4m[:, :, o].T
    W4B[:, :] = w4m[:, :, 4].T

    consts = np.zeros((2, 4), f32)
    consts[:, 0] = bs[3][0]
    consts[:, 1] = bs[3][1]
    consts[:, 2] = EPS

    shared = {
        "TP_BC": TP,
        "TSQ_BC": TSQ,
        "AVEC": AVEC,
        "TCONV": TCONV.astype(f32),
        "W1A": W1A, "W1B": W1B,
        "W2A": W2A, "W2B": W2B,
        "W3A": W3A, "W3B": W3B,
        "W4A": W4A.astype(f16), "W4B": W4B.astype(f16),
        "B2": bs[1][:, None].copy(),
        "B3": bs[2][:, None].copy(),
        "CONSTS": consts,
        "ID2": np.eye(2, dtype=f16),
    }
    in_maps = []
    for c in range(NCORES):
        sl = slice(c * BLOC, (c + 1) * BLOC)
        m = dict(shared)
        m["XS1"] = np.ascontiguousarray(XS1[sl])
        m["XB1"] = np.ascontiguousarray(XB1[sl])
        m["TS2"] = np.ascontiguousarray(TS2[sl])
        m["TB2"] = np.ascontiguousarray(TB2[sl])
        m["XTP"] = np.ascontiguousarray(XTP[sl])
        m["XTSQ"] = np.ascontiguousarray(XTSQ[sl])
        m["PHI"] = np.ascontiguousarray(
            PHI[sl].reshape(BLOC, 128, 16 * 33).astype(f16))
        in_maps.append(m)
    return in_maps


def _get_program():
    struct = _PROG_CACHE["struct"]
    if _PROG_CACHE.get("struct_built") != struct:
        _PROG_CACHE["nc"] = build_program(*struct)
        _PROG_CACHE["struct_built"] = struct
    return _PROG_CACHE["nc"]


def kernel(**inputs):
    from concourse.bass_utils import run_bass_kernel_spmd

    in_maps = make_inmaps(inputs)
    nc = _get_program()
    res = run_bass_kernel_spmd(nc, in_maps, core_ids=list(range(NCORES)))
    outs = [np.asarray(res.results[i]["out"]) for i in range(NCORES)]
    full = np.concatenate(outs, 0)  # [B, 2, T] in sorted-xt column order
    perm_t = _PROG_CACHE["perm_t"]
    out = np.empty((B, T_GRID, 2), np.float32)
    for b in range(B):
        out[b, perm_t[b], 0] = full[b, 0]
        out[b, perm_t[b], 1] = full[b, 1]
    return out


# revision 15
# speedup vs baseline: 2.2674x; 2.2674x over previous
"""ConvCNP1d Trainium2 kernel.

Data-parallel over batch: 16 batches -> 8 cores x 2 batches.

The RBF kernels have support radius ~4.2 units (ls=ln2) on a 128-unit
domain, so K1[x,t] / K2[t,xt] are ~94% exact zeros. The host sorts xc
(with yc) and xt per batch; each 128-row block of sorted points then
only overlaps 1-2 of the four 512-column t/xt chunks. The program is
compiled for the union of needed (block, chunk) pairs across all
batches (same SPMD program on all cores; excluded pairs are < 1e-9
relative). This cuts exp/DVE/matmul work ~2.8x. The decoder output
lands in sorted-xt order and is unpermuted on the host.

Per (block, chunk) pair: the exponent d2 = t'^2 - 2x'*t' is one fused
DVE scalar_tensor_tensor op on a [128,512] tile (t' tables centered
per chunk, baked per column), the a*x'^2 term rides in as the Exp
activation bias, Exp writes the kernel tile in fp16, and one PE matmul
accumulates into the chunk's PSUM tile. The encoder phi has 33 columns
(col 0 = os_psi, col 32 = os_psi*yc) so h0 lands on partition 0 and h1
on partition 32 -- both legal compute-engine bases -- letting the h
epilogue run on PSUM directly: reciprocal_approx_fast on h0 (h0 >= ~6
for this data, eps irrelevant), h0 row scalar-copied straight into
rep2 row 0, ratio row DMA'd into row 1.

conv1d stack as tap-stacked fp16 matmuls: each layer's input is copied
4x on DVE (column-shifted by tap, partition bases 0/32/64/96) into a
shared [128, T] stack tile so taps 0-3 contract in ONE matmul; tap 4
reads the natural tile -- 2 matmuls per 512-chunk instead of 5.
conv1's t-row contribution is host-precomputed (TCONV). conv4 packs mu
at out-col 0 and sigma at out-col 32 (M=33); its sigma softplus
epilogue batches the Ln ops so the act table only swaps twice per
batch. os_rho folds into the K2 exp bias so mu/sigma need no
post-scale.
"""

import numpy as np

T_GRID = 2048
B = 16
N = 2048          # Nc == Nt == 2048
NCORES = 8
BLOC = B // NCORES
EPS = 1e-8
RSUP = 3.75       # RBF support radius in x units: exp(a*3.75^2) ~ 4e-7

_PROG_CACHE = {}


def build_program(enc_lists, dec_lists):
    import concourse.bacc as bacc
    import concourse.tile as tile
    from concourse import mybir

    f32 = mybir.dt.float32
    f16 = mybir.dt.float16
    AF = mybir.ActivationFunctionType
    # Bacc (not raw Bass): its compile() splits multi-sem waits into event
    # semaphores / ldweights, which the TRN2 ISA requires (1 wait per inst).
    nc = bacc.Bacc(None, target_bir_lowering=False)

    TPh = nc.declare_dram_parameter("TP_BC", [1, T_GRID], f32, isOutput=False)
    TSQh = nc.declare_dram_parameter("TSQ_BC", [1, T_GRID], f32, isOutput=False)
    XTPh = nc.declare_dram_parameter("XTP", [BLOC, T_GRID], f32, isOutput=False)
    XTSQh = nc.declare_dram_parameter("XTSQ", [BLOC, T_GRID], f32, isOutput=False)
    TS2h = nc.declare_dram_parameter("TS2", [BLOC, 128, 16, 4], f32, isOutput=False)
    TB2h = nc.declare_dram_parameter("TB2", [BLOC, 128, 16, 4], f32, isOutput=False)
    XS1h = nc.declare_dram_parameter("XS1", [BLOC, 128, 4, 16], f32, isOutput=False)
    XB1h = nc.declare_dram_parameter("XB1", [BLOC, 128, 4, 16], f32, isOutput=False)
    AVh = nc.declare_dram_parameter("AVEC", [128, 2], f32, isOutput=False)
    PHIh = nc.declare_dram_parameter("PHI", [BLOC, 128, 16 * 33], f16, isOutput=False)
    TCh = nc.declare_dram_parameter("TCONV", [16, T_GRID], f32, isOutput=False)
    W1Ah = nc.declare_dram_parameter("W1A", [128, 16], f16, isOutput=False)
    W1Bh = nc.declare_dram_parameter("W1B", [2, 16], f16, isOutput=False)
    W2Ah = nc.declare_dram_parameter("W2A", [128, 32], f16, isOutput=False)
    W2Bh = nc.declare_dram_parameter("W2B", [16, 32], f16, isOutput=False)
    W3Ah = nc.declare_dram_parameter("W3A", [128, 16], f16, isOutput=False)
    W3Bh = nc.declare_dram_parameter("W3B", [32, 16], f16, isOutput=False)
    W4Ah = nc.declare_dram_parameter("W4A", [128, 33], f16, isOutput=False)
    W4Bh = nc.declare_dram_parameter("W4B", [16, 33], f16, isOutput=False)
    B2h = nc.declare_dram_parameter("B2", [32, 1], f32, isOutput=False)
    B3h = nc.declare_dram_parameter("B3", [16, 1], f32, isOutput=False)
    Ch = nc.declare_dram_parameter("CONSTS", [2, 4], f32, isOutput=False)
    ID2h = nc.declare_dram_parameter("ID2", [2, 2], f16, isOutput=False)
    OUTh = nc.declare_dram_parameter("out", [BLOC, 2, T_GRID], f32, isOutput=True)

    with tile.TileContext(nc) as tc:
        with (
            tc.tile_pool(name="singles", bufs=1) as singles,
            tc.tile_pool(name="perb", bufs=2) as perb,
            tc.tile_pool(name="stacks", bufs=2) as stacks,
            tc.tile_pool(name="kpool", bufs=4) as kpool,
            tc.tile_pool(name="small", bufs=1) as small,
            tc.tile_pool(name="outs", bufs=2) as outs,
            tc.tile_pool(name="dvp", bufs=4) as dvp,
            tc.tile_pool(name="psd2", bufs=2, space="PSUM") as psd2,
            tc.tile_pool(name="psacc", bufs=4, space="PSUM") as psacc,
        ):
            import concourse.bass as bass_mod

            def bcast128(dst, src_ap):
                bc = bass_mod.AP(
                    tensor=src_ap.tensor, offset=src_ap.offset,
                    ap=[[0, 128], [1, T_GRID]],
                )
                nc.sync.dma_start(out=dst, in_=bc)

            TP_sb = singles.tile([128, T_GRID], f32)
            bcast128(TP_sb, TPh[:, :])
            TSQ_sb = singles.tile([128, T_GRID], f32)
            bcast128(TSQ_sb, TSQh[:, :])
            AV_sb = singles.tile([128, 2], f32)
            nc.sync.dma_start(out=AV_sb, in_=AVh[:, :])
            TC_sb = singles.tile([16, T_GRID], f32)
            nc.sync.dma_start(out=TC_sb, in_=TCh[:, :])
            W1A_sb = singles.tile([128, 16], f16)
            nc.sync.dma_start(out=W1A_sb, in_=W1Ah[:, :])
            W1B_sb = singles.tile([2, 16], f16)
            nc.sync.dma_start(out=W1B_sb, in_=W1Bh[:, :])
            W2A_sb = singles.tile([128, 32], f16)
            nc.sync.dma_start(out=W2A_sb, in_=W2Ah[:, :])
            W2B_sb = singles.tile([16, 32], f16)
            nc.sync.dma_start(out=W2B_sb, in_=W2Bh[:, :])
            W3A_sb = singles.tile([128, 16], f16)
            nc.sync.dma_start(out=W3A_sb, in_=W3Ah[:, :])
            W3B_sb = singles.tile([32, 16], f16)
            nc.sync.dma_start(out=W3B_sb, in_=W3Bh[:, :])
            W4A_sb = singles.tile([128, 33], f16)
            nc.sync.dma_start(out=W4A_sb, in_=W4Ah[:, :])
            W4B_sb = singles.tile([16, 33], f16)
            nc.sync.dma_start(out=W4B_sb, in_=W4Bh[:, :])
            B2_sb = singles.tile([32, 1], f32)
            nc.sync.dma_start(out=B2_sb, in_=B2h[:, :])
            B3_sb = singles.tile([16, 1], f32)
            nc.sync.dma_start(out=B3_sb, in_=B3h[:, :])
            C_sb = singles.tile([2, 4], f32)
            nc.sync.dma_start(out=C_sb, in_=Ch[:, :])
            ID2_sb = singles.tile([2, 2], f16)
            nc.sync.dma_start(out=ID2_sb, in_=ID2h[:, :])

            st = [dict() for _ in range(BLOC)]  # per-batch tile handles

            def loads(b):
                s = st[b]
                s["XS1"] = perb.tile([128, 4, 16], f32, tag="XS1", name="XS1_sb")
                nc.sync.dma_start(out=s["XS1"], in_=XS1h[b])
                s["XB1"] = perb.tile([128, 4, 16], f32, tag="XB1", name="XB1_sb")
                nc.sync.dma_start(out=s["XB1"], in_=XB1h[b])
                s["TS2"] = perb.tile([128, 16, 4], f32, tag="TS2", name="TS2_sb")
                nc.sync.dma_start(out=s["TS2"], in_=TS2h[b])
                s["TB2"] = perb.tile([128, 16, 4], f32, tag="TB2", name="TB2_sb")
                nc.sync.dma_start(out=s["TB2"], in_=TB2h[b])
                xtp = perb.tile([128, T_GRID], f32, tag="xtp", name="xtp")
                xsrc = XTPh[b]
                nc.sync.dma_start(out=xtp, in_=bass_mod.AP(
                    tensor=xsrc.tensor, offset=xsrc.offset,
                    ap=[[0, 128], [1, T_GRID]]))
                s["xtp"] = xtp
                xtsq = perb.tile([128, T_GRID], f32, tag="xtsq", name="xtsq")
                qsrc = XTSQh[b]
                nc.sync.dma_start(out=xtsq, in_=bass_mod.AP(
                    tensor=qsrc.tensor, offset=qsrc.offset,
                    ap=[[0, 128], [1, T_GRID]]))
                s["xtsq"] = xtsq
                s["PHI"] = perb.tile([128, 16 * 33], f16, tag="PHI", name="PHI_sb")
                nc.sync.dma_start(out=s["PHI"], in_=PHIh[b])
                rep2 = perb.tile([2, T_GRID + 4], f16, tag="rep2", name="rep2")
                nc.vector.memset(rep2[:, 0:2], 0.0)
                nc.vector.memset(rep2[:, T_GRID + 2 : T_GRID + 4], 0.0)
                s["rep2"] = rep2

            def stage_a(b):
                s = st[b]
                h_ps = {}
                kq = []
                seq = []
                for c in range(4):
                    blocks = enc_lists[c]
                    for idx, (i, w0, w1) in enumerate(blocks):
                        seq.append((c, i, w0, w1, idx == 0,
                                    idx == len(blocks) - 1))

                def gen_enc(k):
                    c, i, w0, w1, first, last = seq[k]
                    sl = slice(512 * c + w0, 512 * c + w1)
                    d2s = dvp.tile([128, w1 - w0], f32, tag="d2s", name="d2s")
                    nc.vector.scalar_tensor_tensor(
                        d2s,
                        TP_sb[:, sl],
                        s["XS1"][:, c, i : i + 1],
                        TSQ_sb[:, sl],
                        mybir.AluOpType.mult,
                        mybir.AluOpType.add,
                    )
                    K1 = kpool.tile([128, w1 - w0], f16, tag="K", name="K1")
                    nc.scalar.activation(
                        out=K1, in_=d2s, func=AF.Exp,
                        scale=AV_sb[:, 0:1], bias=s["XB1"][:, c, i : i + 1],
                    )
                    kq.append((K1, c, i, w0, w1, first, last))

                def acc_enc():
                    K1, c, i, w0, w1, first, last = kq.pop(0)
                    if first:
                        h_ps[c] = psacc.tile([33, 512], f32, tag="acc", name="h_acc")
                    nc.tensor.matmul(
                        h_ps[c][:, w0:w1],
                        s["PHI"][:, 33 * i : 33 * i + 33],
                        K1,
                        start=first,
                        stop=last,
                    )
                    if last:
                        # h0 on partition 0, h1 on partition 32; epilogue runs
                        # on PSUM directly. h0 >= ~6 so no eps guard needed
                        # before the reciprocal.
                        sl2 = slice(2 + 512 * c, 2 + 512 * (c + 1))
                        rec = small.tile([1, 512], f32, tag=f"rec{c}", name="rec")
                        ratf = small.tile([1, 512], f16, tag=f"rat{c}", name="ratf")
                        nc.vector.reciprocal_approx_fast(
                            out=rec, in_=h_ps[c][0:1, :])
                        nc.scalar.copy(s["rep2"][0:1, sl2], h_ps[c][0:1, :])
                        nc.vector.tensor_mul(ratf, h_ps[c][32:33, :], rec)
                        nc.sync.dma_start(out=s["rep2"][1:2, sl2], in_=ratf)

                for k in range(len(seq) + 1):
                    if k < len(seq):
                        gen_enc(k)
                    if k >= 1:
                        acc_enc()

            def stack_copies(b, src, nrows, memset_first=False):
                """Copy src rows 4x on DVE (column-shifted by tap o, partition
                base 32*o) into the shared [128, T] stack tile. Layers of a
                batch are serially dependent, so slot rotation across the two
                batches is the only concurrency needed."""
                stk = stacks.tile([128, T_GRID], f16, tag="stk", name="stk")
                if memset_first:
                    # unused partition rows must hold finite values (they get
                    # multiplied by zero weights); pool slots recycle our own
                    # f16 data after conv1, but its first use is raw SBUF.
                    nc.gpsimd.memset(stk, 0.0)
                for o in range(4):
                    nc.vector.tensor_copy(
                        stk[32 * o : 32 * o + nrows, :],
                        src[0:nrows, o : o + T_GRID],
                    )
                return stk

            def stage_b_layer(b, l):
                """conv layer l for batch b: taps 0-3 contract in one K=128
                matmul against the stack tile; tap 4 reads the natural tile."""
                s = st[b]
                if l == 0:
                    for nmt, shp in (("f1", 16), ("f2", 32), ("f3", 16)):
                        s[nmt] = perb.tile([shp, T_GRID + 4], f16, tag=nmt, name=nmt)
                        nc.vector.memset(s[nmt][:, 0:2], 0.0)
                        nc.vector.memset(s[nmt][:, T_GRID + 2 : T_GRID + 4], 0.0)
                    s["fmu"] = perb.tile([1, T_GRID], f16, tag="fmu_r", name="fmu_r")
                    s["fsg"] = perb.tile([1, T_GRID], f16, tag="fsg_r", name="fsg_r")

                if l == 0:
                    # conv1: stack from rep2 (2 rows); the t-row term + b1 is
                    # the host-precomputed TCONV, added on DVE before relu.
                    stk = stack_copies(b, s["rep2"], 2, memset_first=True)
                    for n in range(4):
                        ps = psacc.tile([16, 512], f32, tag="acc", name="c1ps")
                        nc.tensor.matmul(
                            ps, W1A_sb, stk[:, 512 * n : 512 * n + 512],
                            start=True, stop=False,
                        )
                        nc.tensor.matmul(
                            ps, W1B_sb,
                            s["rep2"][:, 512 * n + 4 : 512 * n + 516],
                            start=False, stop=True,
                        )
                        nc.vector.tensor_add(
                            ps, ps, TC_sb[:, 512 * n : 512 * (n + 1)]
                        )
                        nc.scalar.activation(
                            out=s["f1"][:, 2 + 512 * n : 2 + 512 * (n + 1)],
                            in_=ps,
                            func=AF.Relu,
                        )
                elif l in (1, 2):
                    in_tile, wa, wb, bias_sb, O, nrows = (
                        (s["f1"], W2A_sb, W2B_sb, B2_sb, 32, 16) if l == 1
                        else (s["f2"], W3A_sb, W3B_sb, B3_sb, 16, 32)
                    )
                    out_tile = s["f2"] if l == 1 else s["f3"]
                    stk = stack_copies(b, in_tile, nrows)
                    for n in range(4):
                        ps = psacc.tile([O, 512], f32, tag="acc", name="cps")
                        nc.tensor.matmul(
                            ps, wa, stk[:, 512 * n : 512 * n + 512],
                            start=True, stop=False,
                        )
                        nc.tensor.matmul(
                            ps, wb,
                            in_tile[:, 512 * n + 4 : 512 * n + 516],
                            start=False, stop=True,
                        )
                        nc.scalar.activation(
                            out=out_tile[:, 2 + 512 * n : 2 + 512 * (n + 1)],
                            in_=ps,
                            func=AF.Relu,
                            bias=bias_sb,
                        )
                else:
                    # conv4: mu at out-col 0, sigma at out-col 32 (M=33) so one
                    # matmul pair produces both at legal partition bases. The
                    # sigma softplus epilogue batches Exp then Ln across chunks
                    # so the act table only swaps twice per batch:
                    # softplus(x+b) = relu(x+b) + ln(1 + exp(-|x+b|)).
                    stk = stack_copies(b, s["f3"], 16)
                    sas, srs = [], []
                    for n in range(4):
                        ps = psacc.tile([33, 512], f32, tag="acc", name="c4ps")
                        nc.tensor.matmul(
                            ps, W4A_sb, stk[:, 512 * n : 512 * n + 512],
                            start=True, stop=False,
                        )
                        nc.tensor.matmul(
                            ps, W4B_sb,
                            s["f3"][:, 512 * n + 4 : 512 * n + 516],
                            start=False, stop=True,
                        )
                        sl = slice(512 * n, 512 * (n + 1))
                        sa = small.tile([1, 512], f32, tag=f"sa{n}", name="sa")
                        sr = small.tile([1, 512], f32, tag=f"sr{n}", name="sr")
                        nc.scalar.activation(
                            out=s["fmu"][0:1, sl], in_=ps[0:1, :],
                            func=AF.Identity, bias=C_sb[0:1, 0:1],
                        )
                        nc.scalar.activation(
                            out=sa, in_=ps[32:33, :], func=AF.Abs,
                            bias=C_sb[0:1, 1:2],
                        )
                        nc.scalar.activation(
                            out=sr, in_=ps[32:33, :], func=AF.Relu,
                            bias=C_sb[0:1, 1:2],
                        )
                        sas.append(sa)
                        srs.append(sr)
                    for sa in sas:
                        nc.scalar.activation(out=sa, in_=sa, func=AF.Exp, scale=-1.0)
                    for sa in sas:
                        nc.scalar.activation(out=sa, in_=sa, func=AF.Ln, bias=1.0)
                    for n in range(4):
                        sl = slice(512 * n, 512 * (n + 1))
                        nc.gpsimd.tensor_add(s["fsg"][0:1, sl], sas[n], srs[n])

            def stage_t(b):
                # transpose fmu/fsg rows -> fT[p, c, j] = f'_c[128j+p]; the
                # PSUM->SBUF copies alternate scalar/DVE to split the cost.
                s = st[b]
                fT = perb.tile([128, 2, 16], f16, tag="fT", name="fT")
                s["fT"] = fT
                for j in range(16):
                    for c, row in enumerate((s["fmu"], s["fsg"])):
                        tp = psd2.tile([128, 1], f16, tag="d2", name="tp")
                        nc.tensor.transpose(
                            tp, row[0:1, 128 * j : 128 * (j + 1)], ID2_sb[0:1, 0:1]
                        )
                        if (j + c) % 2 == 0:
                            nc.scalar.copy(fT[:, c : c + 1, j], tp)
                        else:
                            nc.vector.tensor_copy(fT[:, c : c + 1, j], tp)

            def stage_c(b):
                s = st[b]
                fT = s["fT"]
                ms_ps = {}
                kq2 = []
                seq = []
                for q in range(4):
                    blocks = dec_lists[q]
                    for idx, (j, w0, w1) in enumerate(blocks):
                        seq.append((q, j, w0, w1, idx == 0,
                                    idx == len(blocks) - 1))

                def gen_dec(k):
                    q, j, w0, w1, first, last = seq[k]
                    sl = slice(512 * q + w0, 512 * q + w1)
                    d2s = dvp.tile([128, w1 - w0], f32, tag="d2s", name="d2c")
                    nc.vector.scalar_tensor_tensor(
                        d2s,
                        s["xtp"][:, sl],
                        s["TS2"][:, j, q : q + 1],
                        s["xtsq"][:, sl],
                        mybir.AluOpType.mult,
                        mybir.AluOpType.add,
                    )
                    K2 = kpool.tile([128, w1 - w0], f16, tag="K", name="K2")
                    nc.scalar.activation(
                        out=K2, in_=d2s, func=AF.Exp,
                        scale=AV_sb[:, 1:2], bias=s["TB2"][:, j, q : q + 1],
                    )
                    kq2.append((K2, q, j, w0, w1, first, last))

                def acc_dec():
                    K2, q, j, w0, w1, first, last = kq2.pop(0)
                    if first:
                        ms_ps[q] = psacc.tile([2, 512], f32, tag="acc", name="ms_acc")
                    nc.tensor.matmul(
                        ms_ps[q][:, w0:w1],
                        fT[:, :, j],
                        K2,
                        start=first,
                        stop=last,
                    )
                    if last:
                        ms_sb = outs.tile([2, 512], f32, tag="ms_sb", name="ms_sb")
                        nc.vector.tensor_copy(ms_sb, ms_ps[q][:, :])
                        nc.sync.dma_start(
                            out=OUTh[b, :, 512 * q : 512 * (q + 1)],
                            in_=ms_sb,
                        )

                for k in range(len(seq) + 1):
                    if k < len(seq):
                        gen_dec(k)
                    if k >= 1:
                        acc_dec()

            loads(0)
            loads(1)
            stage_a(0)
            stage_a(1)
            for l in range(4):
                for b in range(BLOC):
                    stage_b_layer(b, l)
            stage_t(0)
            stage_c(0)
            stage_t(1)
            stage_c(1)

    nc.compile()
    return nc


def make_inmaps(inputs):
    """Host-side sorting, window structure, and table construction.
    Returns (list of 8 per-core input dicts); stores the compiled-program
    structure and output permutations in _PROG_CACHE."""
    f32 = np.float32
    f16 = np.float16
    f64 = np.float64
    xc = np.asarray(inputs["xc"])[..., 0].astype(f32)
    yc = np.asarray(inputs["yc"])[..., 0].astype(f32)
    xt = np.asarray(inputs["xt"])[..., 0].astype(f32)
    ls_psi = f64(np.float32(inputs["ls_psi"]))
    os_psi = f64(np.float32(inputs["os_psi"]))
    ls_rho = f64(np.float32(inputs["ls_rho"]))
    os_rho = f64(np.float32(inputs["os_rho"]))
    w = [np.asarray(inputs[f"w{i}"]).astype(f32) for i in (1, 2, 3, 4)]
    bs = [np.asarray(inputs[f"b{i}"]).astype(f32) for i in (1, 2, 3, 4)]

    lower = np.minimum(xc.min(), xt.min())
    upper = np.maximum(xc.max(), xt.max())
    t64 = np.linspace(f64(lower), f64(upper), T_GRID)

    a_psi = -0.5 / (ls_psi * ls_psi)
    a_rho = -0.5 / (ls_rho * ls_rho)

    # sort context/target points per batch
    perm_c = np.argsort(xc, axis=1)
    perm_t = np.argsort(xt, axis=1)
    xs = np.take_along_axis(xc.astype(f64), perm_c, 1)
    ys = np.take_along_axis(yc.astype(f64), perm_c, 1)
    xts = np.take_along_axis(xt.astype(f64), perm_t, 1)

    # window structure: union over batches of needed (block, chunk) pairs,
    # each with the union of per-batch nonzero column windows. The first
    # pair of each chunk is forced full-width so its start=True matmul
    # initializes the whole PSUM tile.
    enc_wins = [dict() for _ in range(4)]
    dec_wins = [dict() for _ in range(4)]
    for b in range(B):
        for c in range(4):
            tc = t64[512 * c : 512 * (c + 1)]
            for i in range(16):
                blo, bhi = xs[b, 128 * i], xs[b, 128 * i + 127]
                k0 = int(np.searchsorted(tc, blo - RSUP))
                k1 = int(np.searchsorted(tc, bhi + RSUP))
                if k1 > k0:
                    if i in enc_wins[c]:
                        a0, a1 = enc_wins[c][i]
                        enc_wins[c][i] = (min(a0, k0), max(a1, k1))
                    else:
                        enc_wins[c][i] = (k0, k1)
        for q in range(4):
            xq = xts[b, 512 * q : 512 * (q + 1)]
            for j in range(16):
                tlo, thi = t64[128 * j], t64[128 * j + 127]
                k0 = int(np.searchsorted(xq, tlo - RSUP))
                k1 = int(np.searchsorted(xq, thi + RSUP))
                if k1 > k0:
                    if j in dec_wins[q]:
                        a0, a1 = dec_wins[q][j]
                        dec_wins[q][j] = (min(a0, k0), max(a1, k1))
                    else:
                        dec_wins[q][j] = (k0, k1)

    def finalize(wins):
        out = []
        for d in wins:
            lst = []
            for idx, bid in enumerate(sorted(d)):
                if idx == 0:
                    lst.append((bid, 0, 512))
                else:
                    w0, w1 = d[bid]
                    # round to 8-col boundaries, clamp
                    lst.append((bid, max(0, (w0 // 8) * 8),
                                min(512, -(-w1 // 8) * 8)))
            out.append(tuple(lst))
        return tuple(out)

    enc_lists = finalize(enc_wins)
    dec_lists = finalize(dec_wins)
    _PROG_CACHE["struct"] = (enc_lists, dec_lists)
    _PROG_CACHE["perm_t"] = perm_t

    # t' tables centered per 512-chunk, baked per column
    cC = (t64[np.arange(4) * 512] + t64[np.arange(4) * 512 + 511]) / 2
    tp_col = t64 - cC[np.arange(T_GRID) // 512]
    TP = tp_col.astype(f32)[None, :]
    TSQ = (tp_col * tp_col).astype(f32)[None, :]
    AVEC = np.zeros((128, 2), f32)
    AVEC[:, 0] = f32(a_psi)
    AVEC[:, 1] = f32(a_rho)
    ln_os_rho = np.log(os_rho)

    XS1 = np.zeros((B, 128, 4, 16), f32)
    XB1 = np.zeros((B, 128, 4, 16), f32)
    PHI = np.zeros((B, 128, 16, 33), f32)
    TS2 = np.zeros((B, 128, 16, 4), f32)
    TB2 = np.zeros((B, 128, 16, 4), f32)
    XTP = np.zeros((B, T_GRID), f32)
    XTSQ = np.zeros((B, T_GRID), f32)
    tb = t64.reshape(16, 128)                       # [j, p]
    for bi in range(B):
        xcb = xs[bi].reshape(16, 128)               # [i, p]
        for c in range(4):
            xp = xcb - cC[c]
            XS1[bi, :, c, :] = (-2.0 * xp).astype(f32).T
            XB1[bi, :, c, :] = (a_psi * xp * xp).astype(f32).T
        phi_full = np.stack([np.full(N, os_psi), os_psi * ys[bi]], 1)
        pf = phi_full.astype(f32).reshape(16, 128, 2).transpose(1, 0, 2)
        PHI[bi, :, :, 0] = pf[:, :, 0]
        PHI[bi, :, :, 32] = pf[:, :, 1]
        cQ = (xts[bi, np.arange(4) * 512] + xts[bi, np.arange(4) * 512 + 511]) / 2
        for q in range(4):
            tpq = tb - cQ[q]                        # [j, p]
            TS2[bi, :, :, q] = (-2.0 * tpq).astype(f32).T
            TB2[bi, :, :, q] = (a_rho * tpq * tpq + ln_os_rho).astype(f32).T
        xtp_col = xts[bi] - cQ[np.arange(T_GRID) // 512]
        XTP[bi] = xtp_col.astype(f32)
        XTSQ[bi] = (xtp_col * xtp_col).astype(f32)

    # TCONV[o, t] = sum_o' w1[o, 0, o'] * t_pad[t + o'] + b1[o]  (exact fp64)
    t_pad = np.zeros(T_GRID + 4, f64)
    t_pad[2 : 2 + T_GRID] = t64
    TCONV = np.zeros((16, T_GRID), f64)
    for o in range(5):
        TCONV += w[0][:, 0, o].astype(f64)[:, None] * t_pad[o : o + T_GRID][None, :]
    TCONV += bs[0].astype(f64)[:, None]

    def pack_stack(wl, rows):
        # A: [128, O]: partition 32*o + i = wl[j, rows[i], o] for tap o<4
        # B: [I', O]: tap 4
        O, _, _ = wl.shape
        wr = wl[:, rows, :]                         # [O, I', 5]
        I2 = wr.shape[1]
        A = np.zeros((128, O), f32)
        Bm = np.zeros((I2, O), f32)
        for o in range(4):
            A[32 * o : 32 * o + I2, :] = wr[:, :, o].T
        Bm[:, :] = wr[:, :, 4].T
        return A.astype(f16), Bm.astype(f16)

    W1A, W1B = pack_stack(w[0], slice(1, 3))        # I'=2, O=16
    W2A, W2B = pack_stack(w[1], slice(None))        # I'=16, O=32
    W3A, W3B = pack_stack(w[2], slice(None))        # I'=32, O=16
    # conv4: mu weights at out-col 0, sigma at out-col 32
    w4m = np.zeros((33, 16, 5), f32)
    w4m[0] = w[3][0]
    w4m[32] = w[3][1]
    W4A = np.zeros((128, 33), f32)
    W4B = np.zeros((16, 33), f32)
    for o in range(4):
        W4A[32 * o : 32 * o + 16, :] = w4m[:, :, o].T
    W4B[:, :] = w4m[:, :, 4].T

    consts = np.zeros((2, 4), f32)
    consts[:, 0] = bs[3][0]
    consts[:, 1] = bs[3][1]
    consts[:, 2] = EPS

    shared = {
        "TP_BC": TP,
        "TSQ_BC": TSQ,
        "AVEC": AVEC,
        "TCONV": TCONV.astype(f32),
        "W1A": W1A, "W1B": W1B,
        "W2A": W2A, "W2B": W2B,
        "W3A": W3A, "W3B": W3B,
        "W4A": W4A.astype(f16), "W4B": W4B.astype(f16),
        "B2": bs[1][:, None].copy(),
        "B3": bs[2][:, None].copy(),
        "CONSTS": consts,
        "ID2": np.eye(2, dtype=f16),
    }
    in_maps = []
    for c in range(NCORES):
        sl = slice(c * BLOC, (c + 1) * BLOC)
        m = dict(shared)
        m["XS1"] = np.ascontiguousarray(XS1[sl])
        m["XB1"] = np.ascontiguousarray(XB1[sl])
        m["TS2"] = np.ascontiguousarray(TS2[sl])
        m["TB2"] = np.ascontiguousarray(TB2[sl])
        m["XTP"] = np.ascontiguousarray(XTP[sl])
        m["XTSQ"] = np.ascontiguousarray(XTSQ[sl])
        m["PHI"] = np.ascontiguousarray(
            PHI[sl].reshape(BLOC, 128, 16 * 33).astype(f16))
        in_maps.append(m)
    return in_maps


def _get_program():
    struct = _PROG_CACHE["struct"]
    if _PROG_CACHE.get("struct_built") != struct:
        _PROG_CACHE["nc"] = build_program(*struct)
        _PROG_CACHE["struct_built"] = struct
    return _PROG_CACHE["nc"]


def kernel(**inputs):
    from concourse.bass_utils import run_bass_kernel_spmd

    in_maps = make_inmaps(inputs)
    nc = _get_program()
    res = run_bass_kernel_spmd(nc, in_maps, core_ids=list(range(NCORES)))
    outs = [np.asarray(res.results[i]["out"]) for i in range(NCORES)]
    full = np.concatenate(outs, 0)  # [B, 2, T] in sorted-xt column order
    perm_t = _PROG_CACHE["perm_t"]
    out = np.empty((B, T_GRID, 2), np.float32)
    for b in range(B):
        out[b, perm_t[b], 0] = full[b, 0]
        out[b, perm_t[b], 1] = full[b, 1]
    return out


# revision 16
# speedup vs baseline: 2.3457x; 1.0345x over previous
"""ConvCNP1d Trainium2 kernel.

Data-parallel over batch: 16 batches -> 8 cores x 2 batches.

The RBF kernels have support radius ~4.2 units (ls=ln2) on a 128-unit
domain, so K1[x,t] / K2[t,xt] are ~94% exact zeros. The host sorts xc
(with yc) and xt per batch; each 128-row block of sorted points then
only overlaps 1-2 of the four 512-column t/xt chunks. The program is
compiled for the union of needed (block, chunk) pairs across all
batches (same SPMD program on all cores; excluded pairs are < 1e-9
relative). This cuts exp/DVE/matmul work ~2.8x. The decoder output
lands in sorted-xt order and is unpermuted on the host.

Per (block, chunk) pair: the exponent d2 = t'^2 - 2x'*t' is one fused
DVE scalar_tensor_tensor op on a [128,512] tile (t' tables centered
per chunk, baked per column), the a*x'^2 term rides in as the Exp
activation bias, Exp writes the kernel tile in fp16, and one PE matmul
accumulates into the chunk's PSUM tile. The encoder phi has 33 columns
(col 0 = os_psi, col 32 = os_psi*yc) so h0 lands on partition 0 and h1
on partition 32 -- both legal compute-engine bases -- letting the h
epilogue run on PSUM directly: reciprocal_approx_fast on h0 (h0 >= ~6
for this data, eps irrelevant), h0 row scalar-copied straight into
rep2 row 0, ratio row DMA'd into row 1.

conv1d stack as tap-stacked fp16 matmuls: each layer's input is copied
4x on DVE (column-shifted by tap, partition bases 0/32/64/96) into a
shared [128, T] stack tile so taps 0-3 contract in ONE matmul; tap 4
reads the natural tile -- 2 matmuls per 512-chunk instead of 5.
conv1's t-row contribution is host-precomputed (TCONV). conv4 packs mu
at out-col 0 and sigma at out-col 32 (M=33); its sigma softplus
epilogue batches the Ln ops so the act table only swaps twice per
batch. os_rho folds into the K2 exp bias so mu/sigma need no
post-scale.
"""

import numpy as np

T_GRID = 2048
B = 16
N = 2048          # Nc == Nt == 2048
NCORES = 8
BLOC = B // NCORES
EPS = 1e-8
RSUP = 3.75       # RBF support radius in x units: exp(a*3.75^2) ~ 4e-7

_PROG_CACHE = {}


def build_program(enc_lists, dec_lists):
    import concourse.bacc as bacc
    import concourse.tile as tile
    from concourse import mybir

    f32 = mybir.dt.float32
    f16 = mybir.dt.float16
    AF = mybir.ActivationFunctionType
    # Bacc (not raw Bass): its compile() splits multi-sem waits into event
    # semaphores / ldweights, which the TRN2 ISA requires (1 wait per inst).
    nc = bacc.Bacc(None, target_bir_lowering=False)

    TPh = nc.declare_dram_parameter("TP_BC", [1, T_GRID], f32, isOutput=False)
    TSQh = nc.declare_dram_parameter("TSQ_BC", [1, T_GRID], f32, isOutput=False)
    XTPh = nc.declare_dram_parameter("XTP", [BLOC, T_GRID], f32, isOutput=False)
    XTSQh = nc.declare_dram_parameter("XTSQ", [BLOC, T_GRID], f32, isOutput=False)
    TS2h = nc.declare_dram_parameter("TS2", [BLOC, 128, 16, 4], f32, isOutput=False)
    TB2h = nc.declare_dram_parameter("TB2", [BLOC, 128, 16, 4], f32, isOutput=False)
    XS1h = nc.declare_dram_parameter("XS1", [BLOC, 128, 4, 16], f32, isOutput=False)
    XB1h = nc.declare_dram_parameter("XB1", [BLOC, 128, 4, 16], f32, isOutput=False)
    AVh = nc.declare_dram_parameter("AVEC", [128, 2], f32, isOutput=False)
    PHIh = nc.declare_dram_parameter("PHI", [BLOC, 128, 16 * 33], f16, isOutput=False)
    TCh = nc.declare_dram_parameter("TCONV", [16, T_GRID], f32, isOutput=False)
    W1Ah = nc.declare_dram_parameter("W1A", [128, 16], f16, isOutput=False)
    W1Bh = nc.declare_dram_parameter("W1B", [2, 16], f16, isOutput=False)
    W2Ah = nc.declare_dram_parameter("W2A", [128, 32], f16, isOutput=False)
    W2Bh = nc.declare_dram_parameter("W2B", [16, 32], f16, isOutput=False)
    W3Ah = nc.declare_dram_parameter("W3A", [128, 16], f16, isOutput=False)
    W3Bh = nc.declare_dram_parameter("W3B", [32, 16], f16, isOutput=False)
    W4Ah = nc.declare_dram_parameter("W4A", [128, 33], f16, isOutput=False)
    W4Bh = nc.declare_dram_parameter("W4B", [16, 33], f16, isOutput=False)
    B2h = nc.declare_dram_parameter("B2", [32, 1], f32, isOutput=False)
    B3h = nc.declare_dram_parameter("B3", [16, 1], f32, isOutput=False)
    Ch = nc.declare_dram_parameter("CONSTS", [2, 4], f32, isOutput=False)
    ID2h = nc.declare_dram_parameter("ID2", [2, 2], f16, isOutput=False)
    OUTh = nc.declare_dram_parameter("out", [BLOC, 2, T_GRID], f32, isOutput=True)

    with tile.TileContext(nc) as tc:
        with (
            tc.tile_pool(name="singles", bufs=1) as singles,
            tc.tile_pool(name="perb", bufs=2) as perb,
            tc.tile_pool(name="stacks", bufs=2) as stacks,
            tc.tile_pool(name="kpool", bufs=6) as kpool,
            tc.tile_pool(name="small", bufs=1) as small,
            tc.tile_pool(name="outs", bufs=2) as outs,
            tc.tile_pool(name="dvp", bufs=6) as dvp,
            tc.tile_pool(name="psd2", bufs=2, space="PSUM") as psd2,
            tc.tile_pool(name="psacc", bufs=5, space="PSUM") as psacc,
        ):
            import concourse.bass as bass_mod

            def bcast128(dst, src_ap):
                bc = bass_mod.AP(
                    tensor=src_ap.tensor, offset=src_ap.offset,
                    ap=[[0, 128], [1, T_GRID]],
                )
                nc.sync.dma_start(out=dst, in_=bc)

            TP_sb = singles.tile([128, T_GRID], f32)
            bcast128(TP_sb, TPh[:, :])
            TSQ_sb = singles.tile([128, T_GRID], f32)
            bcast128(TSQ_sb, TSQh[:, :])
            AV_sb = singles.tile([128, 2], f32)
            nc.sync.dma_start(out=AV_sb, in_=AVh[:, :])
            TC_sb = singles.tile([16, T_GRID], f32)
            nc.sync.dma_start(out=TC_sb, in_=TCh[:, :])
            W1A_sb = singles.tile([128, 16], f16)
            nc.sync.dma_start(out=W1A_sb, in_=W1Ah[:, :])
            W1B_sb = singles.tile([2, 16], f16)
            nc.sync.dma_start(out=W1B_sb, in_=W1Bh[:, :])
            W2A_sb = singles.tile([128, 32], f16)
            nc.sync.dma_start(out=W2A_sb, in_=W2Ah[:, :])
            W2B_sb = singles.tile([16, 32], f16)
            nc.sync.dma_start(out=W2B_sb, in_=W2Bh[:, :])
            W3A_sb = singles.tile([128, 16], f16)
            nc.sync.dma_start(out=W3A_sb, in_=W3Ah[:, :])
            W3B_sb = singles.tile([32, 16], f16)
            nc.sync.dma_start(out=W3B_sb, in_=W3Bh[:, :])
            W4A_sb = singles.tile([128, 33], f16)
            nc.sync.dma_start(out=W4A_sb, in_=W4Ah[:, :])
            W4B_sb = singles.tile([16, 33], f16)
            nc.sync.dma_start(out=W4B_sb, in_=W4Bh[:, :])
            B2_sb = singles.tile([32, 1], f32)
            nc.sync.dma_start(out=B2_sb, in_=B2h[:, :])
            B3_sb = singles.tile([16, 1], f32)
            nc.sync.dma_start(out=B3_sb, in_=B3h[:, :])
            C_sb = singles.tile([2, 4], f32)
            nc.sync.dma_start(out=C_sb, in_=Ch[:, :])
            ID2_sb = singles.tile([2, 2], f16)
            nc.sync.dma_start(out=ID2_sb, in_=ID2h[:, :])

            st = [dict() for _ in range(BLOC)]  # per-batch tile handles

            def loads(b):
                s = st[b]
                s["XS1"] = perb.tile([128, 4, 16], f32, tag="XS1", name="XS1_sb")
                nc.sync.dma_start(out=s["XS1"], in_=XS1h[b])
                s["XB1"] = perb.tile([128, 4, 16], f32, tag="XB1", name="XB1_sb")
                nc.sync.dma_start(out=s["XB1"], in_=XB1h[b])
                s["TS2"] = perb.tile([128, 16, 4], f32, tag="TS2", name="TS2_sb")
                nc.sync.dma_start(out=s["TS2"], in_=TS2h[b])
                s["TB2"] = perb.tile([128, 16, 4], f32, tag="TB2", name="TB2_sb")
                nc.sync.dma_start(out=s["TB2"], in_=TB2h[b])
                xtp = perb.tile([128, T_GRID], f32, tag="xtp", name="xtp")
                xsrc = XTPh[b]
                nc.sync.dma_start(out=xtp, in_=bass_mod.AP(
                    tensor=xsrc.tensor, offset=xsrc.offset,
                    ap=[[0, 128], [1, T_GRID]]))
                s["xtp"] = xtp
                xtsq = perb.tile([128, T_GRID], f32, tag="xtsq", name="xtsq")
                qsrc = XTSQh[b]
                nc.sync.dma_start(out=xtsq, in_=bass_mod.AP(
                    tensor=qsrc.tensor, offset=qsrc.offset,
                    ap=[[0, 128], [1, T_GRID]]))
                s["xtsq"] = xtsq
                s["PHI"] = perb.tile([128, 16 * 33], f16, tag="PHI", name="PHI_sb")
                nc.sync.dma_start(out=s["PHI"], in_=PHIh[b])
                rep2 = perb.tile([2, T_GRID + 4], f16, tag="rep2", name="rep2")
                nc.vector.memset(rep2[:, 0:2], 0.0)
                nc.vector.memset(rep2[:, T_GRID + 2 : T_GRID + 4], 0.0)
                s["rep2"] = rep2

            def stage_a(b):
                s = st[b]
                h_ps = {}
                kq = []
                seq = []
                for c in range(4):
                    blocks = enc_lists[c]
                    for idx, (i, w0, w1) in enumerate(blocks):
                        seq.append((c, i, w0, w1, idx == 0,
                                    idx == len(blocks) - 1))

                def gen_enc(k):
                    c, i, w0, w1, first, last = seq[k]
                    sl = slice(512 * c + w0, 512 * c + w1)
                    d2s = dvp.tile([128, w1 - w0], f32, tag="d2s", name="d2s")
                    nc.vector.scalar_tensor_tensor(
                        d2s,
                        TP_sb[:, sl],
                        s["XS1"][:, c, i : i + 1],
                        TSQ_sb[:, sl],
                        mybir.AluOpType.mult,
                        mybir.AluOpType.add,
                    )
                    K1 = kpool.tile([128, w1 - w0], f16, tag="K", name="K1")
                    nc.scalar.activation(
                        out=K1, in_=d2s, func=AF.Exp,
                        scale=AV_sb[:, 0:1], bias=s["XB1"][:, c, i : i + 1],
                    )
                    kq.append((K1, c, i, w0, w1, first, last))

                def acc_enc():
                    K1, c, i, w0, w1, first, last = kq.pop(0)
                    if first:
                        h_ps[c] = psacc.tile([33, 512], f32, tag="acc", name="h_acc")
                    nc.tensor.matmul(
                        h_ps[c][:, w0:w1],
                        s["PHI"][:, 33 * i : 33 * i + 33],
                        K1,
                        start=first,
                        stop=last,
                    )
                    if last:
                        # h0 on partition 0, h1 on partition 32; epilogue runs
                        # on PSUM directly. h0 >= ~6 so no eps guard needed
                        # before the reciprocal.
                        sl2 = slice(2 + 512 * c, 2 + 512 * (c + 1))
                        rec = small.tile([1, 512], f32, tag=f"rec{c}", name="rec")
                        ratf = small.tile([1, 512], f16, tag=f"rat{c}", name="ratf")
                        nc.vector.reciprocal_approx_fast(
                            out=rec, in_=h_ps[c][0:1, :])
                        nc.scalar.copy(s["rep2"][0:1, sl2], h_ps[c][0:1, :])
                        nc.vector.tensor_mul(ratf, h_ps[c][32:33, :], rec)
                        nc.sync.dma_start(out=s["rep2"][1:2, sl2], in_=ratf)

                for k in range(len(seq) + 1):
                    if k < len(seq):
                        gen_enc(k)
                    if k >= 1:
                        acc_enc()

            def stack_copies(b, src, nrows, memset_first=False):
                """Copy src rows 4x on DVE (column-shifted by tap o, partition
                base 32*o) into the shared [128, T] stack tile. Layers of a
                batch are serially dependent, so slot rotation across the two
                batches is the only concurrency needed."""
                stk = stacks.tile([128, T_GRID], f16, tag="stk", name="stk")
                if memset_first:
                    # unused partition rows must hold finite values (they get
                    # multiplied by zero weights); pool slots recycle our own
                    # f16 data after conv1, but its first use is raw SBUF.
                    nc.gpsimd.memset(stk, 0.0)
                for o in range(4):
                    nc.vector.tensor_copy(
                        stk[32 * o : 32 * o + nrows, :],
                        src[0:nrows, o : o + T_GRID],
                    )
                return stk

            def stage_b_layer(b, l):
                """conv layer l for batch b: taps 0-3 contract in one K=128
                matmul against the stack tile; tap 4 reads the natural tile."""
                s = st[b]
                if l == 0:
                    for nmt, shp in (("f1", 16), ("f2", 32), ("f3", 16)):
                        s[nmt] = perb.tile([shp, T_GRID + 4], f16, tag=nmt, name=nmt)
                        nc.vector.memset(s[nmt][:, 0:2], 0.0)
                        nc.vector.memset(s[nmt][:, T_GRID + 2 : T_GRID + 4], 0.0)
                    s["fmusg"] = perb.tile([2, T_GRID], f16, tag="fmusg", name="fmusg")
                    s["fsg"] = perb.tile([1, T_GRID], f16, tag="fsg_r", name="fsg_r")

                if l == 0:
                    # conv1: stack from rep2 (2 rows); the t-row term + b1 is
                    # the host-precomputed TCONV, added on DVE before relu.
                    stk = stack_copies(b, s["rep2"], 2, memset_first=True)
                    for n in range(4):
                        ps = psacc.tile([16, 512], f32, tag="acc", name="c1ps")
                        nc.tensor.matmul(
                            ps, W1A_sb, stk[:, 512 * n : 512 * n + 512],
                            start=True, stop=False,
                        )
                        nc.tensor.matmul(
                            ps, W1B_sb,
                            s["rep2"][:, 512 * n + 4 : 512 * n + 516],
                            start=False, stop=True,
                        )
                        nc.vector.tensor_add(
                            ps, ps, TC_sb[:, 512 * n : 512 * (n + 1)]
                        )
                        nc.scalar.activation(
                            out=s["f1"][:, 2 + 512 * n : 2 + 512 * (n + 1)],
                            in_=ps,
                            func=AF.Relu,
                        )
                elif l in (1, 2):
                    in_tile, wa, wb, bias_sb, O, nrows = (
                        (s["f1"], W2A_sb, W2B_sb, B2_sb, 32, 16) if l == 1
                        else (s["f2"], W3A_sb, W3B_sb, B3_sb, 16, 32)
                    )
                    out_tile = s["f2"] if l == 1 else s["f3"]
                    stk = stack_copies(b, in_tile, nrows)
                    for n in range(4):
                        ps = psacc.tile([O, 512], f32, tag="acc", name="cps")
                        nc.tensor.matmul(
                            ps, wa, stk[:, 512 * n : 512 * n + 512],
                            start=True, stop=False,
                        )
                        nc.tensor.matmul(
                            ps, wb,
                            in_tile[:, 512 * n + 4 : 512 * n + 516],
                            start=False, stop=True,
                        )
                        nc.scalar.activation(
                            out=out_tile[:, 2 + 512 * n : 2 + 512 * (n + 1)],
                            in_=ps,
                            func=AF.Relu,
                            bias=bias_sb,
                        )
                else:
                    # conv4: mu at out-col 0, sigma at out-col 32 (M=33) so one
                    # matmul pair produces both at legal partition bases. The
                    # sigma softplus epilogue batches Exp then Ln across chunks
                    # so the act table only swaps twice per batch:
                    # softplus(x+b) = relu(x+b) + ln(1 + exp(-|x+b|)).
                    stk = stack_copies(b, s["f3"], 16)
                    sas, srs = [], []
                    for n in range(4):
                        ps = psacc.tile([33, 512], f32, tag="acc", name="c4ps")
                        nc.tensor.matmul(
                            ps, W4A_sb, stk[:, 512 * n : 512 * n + 512],
                            start=True, stop=False,
                        )
                        nc.tensor.matmul(
                            ps, W4B_sb,
                            s["f3"][:, 512 * n + 4 : 512 * n + 516],
                            start=False, stop=True,
                        )
                        sl = slice(512 * n, 512 * (n + 1))
                        sa = small.tile([1, 512], f32, tag=f"sa{n}", name="sa")
                        sr = small.tile([1, 512], f32, tag=f"sr{n}", name="sr")
                        nc.scalar.activation(
                            out=s["fmusg"][0:1, sl], in_=ps[0:1, :],
                            func=AF.Identity, bias=C_sb[0:1, 0:1],
                        )
                        nc.scalar.activation(
                            out=sa, in_=ps[32:33, :], func=AF.Abs,
                            bias=C_sb[0:1, 1:2],
                        )
                        nc.scalar.activation(
                            out=sr, in_=ps[32:33, :], func=AF.Relu,
                            bias=C_sb[0:1, 1:2],
                        )
                        sas.append(sa)
                        srs.append(sr)
                    for sa in sas:
                        nc.scalar.activation(out=sa, in_=sa, func=AF.Exp, scale=-1.0)
                    for sa in sas:
                        nc.scalar.activation(out=sa, in_=sa, func=AF.Ln, bias=1.0)
                    for n in range(4):
                        sl = slice(512 * n, 512 * (n + 1))
                        nc.vector.tensor_add(s["fsg"][0:1, sl], sas[n], srs[n])
                    # sigma row to partition 1 (compute engines cannot write
                    # base 1; DMA has no base restriction)
                    nc.sync.dma_start(out=s["fmusg"][1:2, :], in_=s["fsg"])

            def stage_t(b):
                # transpose fmusg [2, 128]-chunks -> fT[p, c, j] = f'_c[128j+p]
                # in one PE op per j; PSUM->SBUF copies alternate scalar/DVE.
                s = st[b]
                fT = perb.tile([128, 2, 16], f16, tag="fT", name="fT")
                s["fT"] = fT
                for j in range(16):
                    tp = psd2.tile([128, 2], f16, tag="d2", name="tp")
                    nc.tensor.transpose(
                        tp, s["fmusg"][:, 128 * j : 128 * (j + 1)], ID2_sb
                    )
                    if j % 2 == 0:
                        nc.scalar.copy(fT[:, :, j], tp)
                    else:
                        nc.vector.tensor_copy(fT[:, :, j], tp)

            def stage_c(b):
                s = st[b]
                fT = s["fT"]
                ms_ps = {}
                kq2 = []
                seq = []
                for q in range(4):
                    blocks = dec_lists[q]
                    for idx, (j, w0, w1) in enumerate(blocks):
                        seq.append((q, j, w0, w1, idx == 0,
                                    idx == len(blocks) - 1))

                def gen_dec(k):
                    q, j, w0, w1, first, last = seq[k]
                    sl = slice(512 * q + w0, 512 * q + w1)
                    d2s = dvp.tile([128, w1 - w0], f32, tag="d2s", name="d2c")
                    nc.vector.scalar_tensor_tensor(
                        d2s,
                        s["xtp"][:, sl],
                        s["TS2"][:, j, q : q + 1],
                        s["xtsq"][:, sl],
                        mybir.AluOpType.mult,
                        mybir.AluOpType.add,
                    )
                    K2 = kpool.tile([128, w1 - w0], f16, tag="K", name="K2")
                    nc.scalar.activation(
                        out=K2, in_=d2s, func=AF.Exp,
                        scale=AV_sb[:, 1:2], bias=s["TB2"][:, j, q : q + 1],
                    )
                    kq2.append((K2, q, j, w0, w1, first, last))

                def acc_dec():
                    K2, q, j, w0, w1, first, last = kq2.pop(0)
                    if first:
                        ms_ps[q] = psacc.tile([2, 512], f32, tag="acc", name="ms_acc")
                    nc.tensor.matmul(
                        ms_ps[q][:, w0:w1],
                        fT[:, :, j],
                        K2,
                        start=first,
                        stop=last,
                    )
                    if last:
                        ms_sb = outs.tile([2, 512], f32, tag="ms_sb", name="ms_sb")
                        nc.vector.tensor_copy(ms_sb, ms_ps[q][:, :])
                        nc.sync.dma_start(
                            out=OUTh[b, :, 512 * q : 512 * (q + 1)],
                            in_=ms_sb,
                        )

                for k in range(len(seq) + 1):
                    if k < len(seq):
                        gen_dec(k)
                    if k >= 1:
                        acc_dec()

            loads(0)
            loads(1)
            stage_a(0)
            stage_a(1)
            for l in range(4):
                for b in range(BLOC):
                    stage_b_layer(b, l)
            stage_t(0)
            stage_c(0)
            stage_t(1)
            stage_c(1)

    nc.compile()
    return nc


def make_inmaps(inputs):
    """Host-side sorting, window structure, and table construction.
    Returns (list of 8 per-core input dicts); stores the compiled-program
    structure and output permutations in _PROG_CACHE."""
    f32 = np.float32
    f16 = np.float16
    f64 = np.float64
    xc = np.asarray(inputs["xc"])[..., 0].astype(f32)
    yc = np.asarray(inputs["yc"])[..., 0].astype(f32)
    xt = np.asarray(inputs["xt"])[..., 0].astype(f32)
    ls_psi = f64(np.float32(inputs["ls_psi"]))
    os_psi = f64(np.float32(inputs["os_psi"]))
    ls_rho = f64(np.float32(inputs["ls_rho"]))
    os_rho = f64(np.float32(inputs["os_rho"]))
    w = [np.asarray(inputs[f"w{i}"]).astype(f32) for i in (1, 2, 3, 4)]
    bs = [np.asarray(inputs[f"b{i}"]).astype(f32) for i in (1, 2, 3, 4)]

    lower = np.minimum(xc.min(), xt.min())
    upper = np.maximum(xc.max(), xt.max())
    t64 = np.linspace(f64(lower), f64(upper), T_GRID)

    a_psi = -0.5 / (ls_psi * ls_psi)
    a_rho = -0.5 / (ls_rho * ls_rho)

    # sort context/target points per batch
    perm_c = np.argsort(xc, axis=1)
    perm_t = np.argsort(xt, axis=1)
    xs = np.take_along_axis(xc.astype(f64), perm_c, 1)
    ys = np.take_along_axis(yc.astype(f64), perm_c, 1)
    xts = np.take_along_axis(xt.astype(f64), perm_t, 1)

    # window structure: union over batches of needed (block, chunk) pairs,
    # each with the union of per-batch nonzero column windows. The first
    # pair of each chunk is forced full-width so its start=True matmul
    # initializes the whole PSUM tile.
    enc_wins = [dict() for _ in range(4)]
    dec_wins = [dict() for _ in range(4)]
    for b in range(B):
        for c in range(4):
            tc = t64[512 * c : 512 * (c + 1)]
            for i in range(16):
                blo, bhi = xs[b, 128 * i], xs[b, 128 * i + 127]
                k0 = int(np.searchsorted(tc, blo - RSUP))
                k1 = int(np.searchsorted(tc, bhi + RSUP))
                if k1 > k0:
                    if i in enc_wins[c]:
                        a0, a1 = enc_wins[c][i]
                        enc_wins[c][i] = (min(a0, k0), max(a1, k1))
                    else:
                        enc_wins[c][i] = (k0, k1)
        for q in range(4):
            xq = xts[b, 512 * q : 512 * (q + 1)]
            for j in range(16):
                tlo, thi = t64[128 * j], t64[128 * j + 127]
                k0 = int(np.searchsorted(xq, tlo - RSUP))
                k1 = int(np.searchsorted(xq, thi + RSUP))
                if k1 > k0:
                    if j in dec_wins[q]:
                        a0, a1 = dec_wins[q][j]
                        dec_wins[q][j] = (min(a0, k0), max(a1, k1))
                    else:
                        dec_wins[q][j] = (k0, k1)

    def finalize(wins):
        out = []
        for d in wins:
            lst = []
            for idx, bid in enumerate(sorted(d)):
                if idx == 0:
                    lst.append((bid, 0, 512))
                else:
                    w0, w1 = d[bid]
                    # round to 8-col boundaries, clamp
                    lst.append((bid, max(0, (w0 // 8) * 8),
                                min(512, -(-w1 // 8) * 8)))
            out.append(tuple(lst))
        return tuple(out)

    enc_lists = finalize(enc_wins)
    dec_lists = finalize(dec_wins)
    _PROG_CACHE["struct"] = (enc_lists, dec_lists)
    _PROG_CACHE["perm_t"] = perm_t

    # t' tables centered per 512-chunk, baked per column
    cC = (t64[np.arange(4) * 512] + t64[np.arange(4) * 512 + 511]) / 2
    tp_col = t64 - cC[np.arange(T_GRID) // 512]
    TP = tp_col.astype(f32)[None, :]
    TSQ = (tp_col * tp_col).astype(f32)[None, :]
    AVEC = np.zeros((128, 2), f32)
    AVEC[:, 0] = f32(a_psi)
    AVEC[:, 1] = f32(a_rho)
    ln_os_rho = np.log(os_rho)

    XS1 = np.zeros((B, 128, 4, 16), f32)
    XB1 = np.zeros((B, 128, 4, 16), f32)
    PHI = np.zeros((B, 128, 16, 33), f32)
    TS2 = np.zeros((B, 128, 16, 4), f32)
    TB2 = np.zeros((B, 128, 16, 4), f32)
    XTP = np.zeros((B, T_GRID), f32)
    XTSQ = np.zeros((B, T_GRID), f32)
    tb = t64.reshape(16, 128)                       # [j, p]
    for bi in range(B):
        xcb = xs[bi].reshape(16, 128)               # [i, p]
        for c in range(4):
            xp = xcb - cC[c]
            XS1[bi, :, c, :] = (-2.0 * xp).astype(f32).T
            XB1[bi, :, c, :] = (a_psi * xp * xp).astype(f32).T
        phi_full = np.stack([np.full(N, os_psi), os_psi * ys[bi]], 1)
        pf = phi_full.astype(f32).reshape(16, 128, 2).transpose(1, 0, 2)
        PHI[bi, :, :, 0] = pf[:, :, 0]
        PHI[bi, :, :, 32] = pf[:, :, 1]
        cQ = (xts[bi, np.arange(4) * 512] + xts[bi, np.arange(4) * 512 + 511]) / 2
        for q in range(4):
            tpq = tb - cQ[q]                        # [j, p]
            TS2[bi, :, :, q] = (-2.0 * tpq).astype(f32).T
            TB2[bi, :, :, q] = (a_rho * tpq * tpq + ln_os_rho).astype(f32).T
        xtp_col = xts[bi] - cQ[np.arange(T_GRID) // 512]
        XTP[bi] = xtp_col.astype(f32)
        XTSQ[bi] = (xtp_col * xtp_col).astype(f32)

    # TCONV[o, t] = sum_o' w1[o, 0, o'] * t_pad[t + o'] + b1[o]  (exact fp64)
    t_pad = np.zeros(T_GRID + 4, f64)
    t_pad[2 : 2 + T_GRID] = t64
    TCONV = np.zeros((16, T_GRID), f64)
    for o in range(5):
        TCONV += w[0][:, 0, o].astype(f64)[:, None] * t_pad[o : o + T_GRID][None, :]
    TCONV += bs[0].astype(f64)[:, None]

    def pack_stack(wl, rows):
        # A: [128, O]: partition 32*o + i = wl[j, rows[i], o] for tap o<4
        # B: [I', O]: tap 4
        O, _, _ = wl.shape
        wr = wl[:, rows, :]                         # [O, I', 5]
        I2 = wr.shape[1]
        A = np.zeros((128, O), f32)
        Bm = np.zeros((I2, O), f32)
        for o in range(4):
            A[32 * o : 32 * o + I2, :] = wr[:, :, o].T
        Bm[:, :] = wr[:, :, 4].T
        return A.astype(f16), Bm.astype(f16)

    W1A, W1B = pack_stack(w[0], slice(1, 3))        # I'=2, O=16
    W2A, W2B = pack_stack(w[1], slice(None))        # I'=16, O=32
    W3A, W3B = pack_stack(w[2], slice(None))        # I'=32, O=16
    # conv4: mu weights at out-col 0, sigma at out-col 32
    w4m = np.zeros((33, 16, 5), f32)
    w4m[0] = w[3][0]
    w4m[32] = w[3][1]
    W4A = np.zeros((128, 33), f32)
    W4B = np.zeros((16, 33), f32)
    for o in range(4):
        W4A[32 * o : 32 * o + 16, :] = w4m[:, :, o].T
    W4B[:, :] = w4m[:, :, 4].T

    consts = np.zeros((2, 4), f32)
    consts[:, 0] = bs[3][0]
    consts[:, 1] = bs[3][1]
    consts[:, 2] = EPS

    shared = {
        "TP_BC": TP,
        "TSQ_BC": TSQ,
        "AVEC": AVEC,
        "TCONV": TCONV.astype(f32),
        "W1A": W1A, "W1B": W1B,
        "W2A": W2A, "W2B": W2B,
        "W3A": W3A, "W3B": W3B,
        "W4A": W4A.astype(f16), "W4B": W4B.astype(f16),
        "B2": bs[1][:, None].copy(),
        "B3": bs[2][:, None].copy(),
        "CONSTS": consts,
        "ID2": np.eye(2, dtype=f16),
    }
    in_maps = []
    for c in range(NCORES):
        sl = slice(c * BLOC, (c + 1) * BLOC)
        m = dict(shared)
        m["XS1"] = np.ascontiguousarray(XS1[sl])
        m["XB1"] = np.ascontiguousarray(XB1[sl])
        m["TS2"] = np.ascontiguousarray(TS2[sl])
        m["TB2"] = np.ascontiguousarray(TB2[sl])
        m["XTP"] = np.ascontiguousarray(XTP[sl])
        m["XTSQ"] = np.ascontiguousarray(XTSQ[sl])
        m["PHI"] = np.ascontiguousarray(
            PHI[sl].reshape(BLOC, 128, 16 * 33).astype(f16))
        in_maps.append(m)
    return in_maps


def _get_program():
    struct = _PROG_CACHE["struct"]
    if _PROG_CACHE.get("struct_built") != struct:
        _PROG_CACHE["nc"] = build_program(*struct)
        _PROG_CACHE["struct_built"] = struct
    return _PROG_CACHE["nc"]


def kernel(**inputs):
    from concourse.bass_utils import run_bass_kernel_spmd

    in_maps = make_inmaps(inputs)
    nc = _get_program()
    res = run_bass_kernel_spmd(nc, in_maps, core_ids=list(range(NCORES)))
    outs = [np.asarray(res.results[i]["out"]) for i in range(NCORES)]
    full = np.concatenate(outs, 0)  # [B, 2, T] in sorted-xt column order
    perm_t = _PROG_CACHE["perm_t"]
    out = np.empty((B, T_GRID, 2), np.float32)
    for b in range(B):
        out[b, perm_t[b], 0] = full[b, 0]
        out[b, perm_t[b], 1] = full[b, 1]
    return out
